# revision 17
# baseline (speedup 1.0000x reference)
"""Dictionary cross-attention kernel for trn2 (8 NeuronCores, batch-parallel).

Math (per batch):
  q = x @ Wq^T + bq                    [n, 10]
  kn = l2norm(td @ Wk^T + bk)          [64, 10]
  v  = td @ Wv^T + bv                  [64, 192]
  logits = s * (q/||q||) @ kn^T,  s = 1 + clip(scale,0,3)*ln(64)
  attn = softmax(logits); out = attn @ v

Device-side tricks:
- Fold q projection + key dot-product into ONE matmul over x:
  G = x @ B + bias_row, B = [Wq^T @ (s*kn)^T | Wq^T]  ([192, 74]).
  G[:, :64] = scaled unnormalized logits, G[:, 64:74] = q (for ||q||).
  No softmax max-subtraction needed: |logits| <= s ~ 5.16.
- rsqrt via Newton on the vector engine (scalar engine stays in the
  exp_and_others activation-table set -> zero table reloads).
"""

import os
import numpy as np

N, C, M, DR = 16384, 192, 64, 10
B_FULL = 8
GCOLS = M + DR  # 74
NB = 512        # rows per block
NBLK = N // NB  # 32
NJ = NB // 128  # 4 row-groups per block
NUM_TOKENS = 64
EPS = 1e-12

_cache = {}


def _build_bass():
    import concourse.bass as bass
    import concourse.bacc as bacc
    import concourse.tile as tile
    import concourse.mybir as mybir

    f32 = mybir.dt.float32
    i32 = mybir.dt.int32
    AF = mybir.ActivationFunctionType
    ALU = mybir.AluOpType
    ts = bass.ts

    nc = bacc.Bacc()
    x_d = nc.dram_tensor("x", [N, C], f32, kind="ExternalInput")
    b_d = nc.dram_tensor("bmat", [C + 1, GCOLS], f32, kind="ExternalInput")
    v_d = nc.dram_tensor("vmat", [M, C], f32, kind="ExternalInput")
    i_d = nc.dram_tensor("ident", [128, 128], f32, kind="ExternalInput")
    out_d = nc.dram_tensor("out", [N, C], f32, kind="ExternalOutput")
    attn_d = nc.dram_tensor("attn", [N, M], f32, kind="ExternalOutput")

    xr = x_d[:].rearrange("(i j p) c -> i p j c", j=NJ, p=128)
    outr = out_d[:].rearrange("(i j p) c -> i p j c", j=NJ, p=128)
    attnr = attn_d[:].rearrange("(i j p) m -> i p j m", j=NJ, p=128)

    with tile.TileContext(nc) as tc:
        with (
            tc.tile_pool(name="const", bufs=1) as pc,
            tc.tile_pool(name="xin", bufs=3) as px,
            tc.tile_pool(name="xt0", bufs=3) as pxt0,
            tc.tile_pool(name="xt1", bufs=3) as pxt1,
            tc.tile_pool(name="esb", bufs=3) as pe,
            tc.tile_pool(name="atn", bufs=3) as pa,
            tc.tile_pool(name="ats", bufs=3) as pat,
            tc.tile_pool(name="osb", bufs=3) as po,
            tc.tile_pool(name="stat", bufs=2) as pst,
            tc.tile_pool(name="junk", bufs=2) as pjk,
            tc.tile_pool(name="ppt0", bufs=2, space="PSUM") as ppt0,
            tc.tile_pool(name="p64", bufs=2, space="PSUM") as p64,
            tc.tile_pool(name="ppg", bufs=2, space="PSUM") as ppg,
            tc.tile_pool(name="ppo", bufs=2, space="PSUM") as ppo,
        ):
            ident = pc.tile([128, 128], f32)
            nc.sync.dma_start(ident[:], i_d[:])
            b0 = pc.tile([128, GCOLS], f32)
            nc.sync.dma_start(b0[:], b_d[0:128, :])
            b1 = pc.tile([65, GCOLS], f32)
            nc.sync.dma_start(b1[:], b_d[128 : C + 1, :])
            vsb = pc.tile([M, C], f32)
            nc.sync.dma_start(vsb[:], v_d[:])

            def phase_load(i):
                x_sb = px.tile([128, NJ, C], f32)
                nc.sync.dma_start(x_sb[:], xr[i])
                return x_sb

            def phase_transpose(x_sb):
                pt0 = ppt0.tile([128, NB], f32)
                pt1 = p64.tile([64, NB], f32, tag="t64")
                for j in range(NJ):
                    nc.tensor.transpose(pt0[:, ts(j, 128)], x_sb[:, j, 0:128], ident[:])
                    nc.tensor.transpose(pt1[:, ts(j, 128)], x_sb[:, j, 128:C], ident[:])
                xt0 = pxt0.tile([128, NB], f32)
                nc.scalar.copy(xt0[:], pt0[:])
                xt1 = pxt1.tile([65, NB], f32)
                nc.vector.tensor_copy(xt1[0:64, :], pt1[:])
                nc.vector.memset(xt1[64:65, :], 1.0)
                return xt0, xt1

            def phase_g(xt0, xt1):
                pg = ppg.tile([128, NJ, GCOLS], f32)
                for j in range(NJ):
                    nc.tensor.matmul(
                        pg[:, j, :], xt0[:, ts(j, 128)], b0[:], start=True, stop=False
                    )
                    nc.tensor.matmul(
                        pg[:, j, :], xt1[:, ts(j, 128)], b1[:], start=False, stop=True
                    )
                return pg

            def phase_softmax(i, pg):
                qsq = pjk.tile([128, NJ, DR], f32)
                nc.scalar.activation(qsq[:], pg[:, :, M:GCOLS], AF.Square)
                ss = pst.tile([128, NJ], f32)
                nc.vector.tensor_reduce(
                    ss[:], qsq[:], axis=mybir.AxisListType.X, op=ALU.add
                )
                ssm = pst.tile([128, NJ], f32)
                hn = pst.tile([128, NJ], f32)
                nc.vector.tensor_scalar_max(ssm[:], ss[:], EPS * EPS)
                nc.vector.tensor_scalar(hn[:], ssm[:], -0.5, None, ALU.mult)
                u = pst.tile([128, NJ], i32)
                nc.vector.tensor_scalar(
                    u[:], ssm[:].bitcast(i32), 1, None, ALU.logical_shift_right
                )
                y = pst.tile([128, NJ], f32)
                nc.vector.tensor_scalar(
                    y[:].bitcast(i32), u[:], -1, 0x5F3759DF, ALU.mult, ALU.add
                )
                for _ in range(2):
                    a = pst.tile([128, NJ], f32)
                    nc.vector.tensor_tensor(a[:], y[:], y[:], ALU.mult)
                    b = pst.tile([128, NJ], f32)
                    nc.vector.tensor_tensor(b[:], a[:], hn[:], ALU.mult)
                    y2 = pst.tile([128, NJ], f32)
                    nc.vector.scalar_tensor_tensor(
                        y2[:], b[:], 1.5, y[:], ALU.add, ALU.mult
                    )
                    y = y2
                rsc = y
                E = pe.tile([128, NJ, M], f32)
                den = pst.tile([128, NJ], f32)
                for j in range(NJ):
                    nc.scalar.activation(
                        E[:, j, :],
                        pg[:, j, 0:M],
                        AF.Exp,
                        scale=rsc[:, j : j + 1],
                        accum_out=den[:, j : j + 1],
                    )
                rden = pst.tile([128, NJ], f32)
                nc.vector.reciprocal(rden[:], den[:])
                attn_sb = pa.tile([128, NJ, M], f32)
                for j in range(NJ):
                    nc.vector.tensor_scalar_mul(
                        attn_sb[:, j, :], E[:, j, :], rden[:, j : j + 1]
                    )
                nc.sync.dma_start(attnr[i], attn_sb[:])
                return E, rden

            def phase_et(E):
                pat_ps = p64.tile([64, NB], f32, tag="t64")
                for j in range(NJ):
                    nc.tensor.transpose(pat_ps[:, ts(j, 128)], E[:, j, :], ident[:])
                at_sb = pat.tile([64, NB], f32)
                nc.scalar.copy(at_sb[:], pat_ps[:])
                return at_sb

            def phase_av(i, at_sb, rden):
                o_sb = po.tile([128, NJ, C], f32)
                for j in range(NJ):
                    po_ps = ppo.tile([128, C], f32)
                    nc.tensor.matmul(
                        po_ps[:], at_sb[:, ts(j, 128)], vsb[:], start=True, stop=True
                    )
                    nc.vector.tensor_scalar_mul(
                        o_sb[:, j, :], po_ps[:], rden[:, j : j + 1]
                    )
                nc.sync.dma_start(outr[i], o_sb[:])

            # pair-batched: long G runs keep the PE MAC array continuously
            # active (HAM warm); transposes are batched likewise.
            for i2 in range(NBLK // 2):
                iA, iB = 2 * i2, 2 * i2 + 1
                xA = phase_load(iA)
                xB = phase_load(iB)
                xtA = phase_transpose(xA)
                xtB = phase_transpose(xB)
                pgA = phase_g(*xtA)
                pgB = phase_g(*xtB)
                EA, rdA = phase_softmax(iA, pgA)
                EB, rdB = phase_softmax(iB, pgB)
                atA = phase_et(EA)
                atB = phase_et(EB)
                phase_av(iA, atA, rdA)
                phase_av(iB, atB, rdB)
    nc.finalize()
    return nc


def _host_prep(td, wq_w, wq_b, wk_w, wk_b, wv_w, wv_b, scale):
    """Per-batch B matrix [193, 74] and v [64, 192], all float32."""
    s = 1.0 + np.clip(scale.astype(np.float32), 0.0, 3.0) * np.float32(
        np.log(NUM_TOKENS)
    )
    s = np.float32(s.reshape(-1)[0])
    bmats, vmats = [], []
    for b in range(td.shape[0]):
        k = td[b] @ wk_w.T + wk_b  # [64, 10]
        knrm = np.sqrt((k * k).sum(-1, keepdims=True))
        kn = k / np.maximum(knrm, EPS)
        skn = s * kn  # fold temperature into keys
        A = wq_w.T @ skn.T  # [192, 64]
        d = skn @ wq_b  # [64]
        Bm = np.zeros((C + 1, GCOLS), dtype=np.float32)
        Bm[0:C, 0:M] = A
        Bm[0:C, M:GCOLS] = wq_w.T
        Bm[C, 0:M] = d
        Bm[C, M:GCOLS] = wq_b
        bmats.append(Bm)
        vmats.append((td[b] @ wv_w.T + wv_b).astype(np.float32))
    return np.stack(bmats), np.stack(vmats)


def kernel(x, td, wq_w, wq_b, wk_w, wk_b, wv_w, wv_b, scale, h, w):
    from concourse.bass_utils import run_bass_kernel_spmd

    x = np.ascontiguousarray(np.asarray(x, dtype=np.float32))
    td = np.asarray(td, dtype=np.float32)
    wq_w = np.asarray(wq_w, dtype=np.float32)
    wq_b = np.asarray(wq_b, dtype=np.float32)
    wk_w = np.asarray(wk_w, dtype=np.float32)
    wk_b = np.asarray(wk_b, dtype=np.float32)
    wv_w = np.asarray(wv_w, dtype=np.float32)
    wv_b = np.asarray(wv_b, dtype=np.float32)
    scale = np.asarray(scale, dtype=np.float32)

    bmats, vmats = _host_prep(td, wq_w, wq_b, wk_w, wk_b, wv_w, wv_b, scale)
    ident = np.eye(128, dtype=np.float32)

    if "nc" not in _cache:
        _cache["nc"] = _build_bass()
    nc = _cache["nc"]

    in_maps = [
        {
            "x": np.ascontiguousarray(x[b]),
            "bmat": np.ascontiguousarray(bmats[b]),
            "vmat": np.ascontiguousarray(vmats[b]),
            "ident": ident,
        }
        for b in range(B_FULL)
    ]
    trace = os.environ.get("KERNEL_TRACE", "0") == "1"
    res = run_bass_kernel_spmd(nc, in_maps, core_ids=list(range(B_FULL)), trace=trace)
    kernel._last_exec_ns = res.exec_time_ns
    kernel._last_results = res
    out = np.stack([r["out"] for r in res.results]).reshape(B_FULL, N, C)
    attn = np.stack([r["attn"] for r in res.results]).reshape(B_FULL, N, M)
    return out, attn


kernel._last_exec_ns = None
kernel._last_results = None


# revision 18
# speedup vs baseline: 1.7707x; 1.7707x over previous
"""Dictionary cross-attention kernel for trn2 (8 NeuronCores, batch-parallel).

Math (per batch):
  q = x @ Wq^T + bq                    [n, 10]
  kn = l2norm(td @ Wk^T + bk)          [64, 10]
  v  = td @ Wv^T + bv                  [64, 192]
  logits = s * (q/||q||) @ kn^T,  s = 1 + clip(scale,0,3)*ln(64)
  attn = softmax(logits); out = attn @ v

Device-side tricks:
- Fold q projection + key dot-product into ONE matmul over x:
  G = x @ B + bias_row, B = [Wq^T @ (s*kn)^T | Wq^T]  ([192, 74]).
  G[:, :64] = scaled unnormalized logits, G[:, 64:74] = q (for ||q||).
  No softmax max-subtraction needed: |logits| <= s ~ 5.16.
- rsqrt via Newton on the vector engine (scalar engine stays in the
  exp_and_others activation-table set -> zero table reloads).
"""

import os
import numpy as np

N, C, M, DR = 16384, 192, 64, 10
B_FULL = 8
GCOLS = M + DR  # 74
NB = 512        # rows per block
NBLK = N // NB  # 32
NJ = NB // 128  # 4 row-groups per block
NUM_TOKENS = 64
EPS = 1e-12

_cache = {}


def _build_bass():
    import concourse.bass as bass
    import concourse.bacc as bacc
    import concourse.tile as tile
    import concourse.mybir as mybir

    f32 = mybir.dt.float32
    i32 = mybir.dt.int32
    AF = mybir.ActivationFunctionType
    ALU = mybir.AluOpType
    ts = bass.ts

    nc = bacc.Bacc()
    x_d = nc.dram_tensor("x", [N, C], f32, kind="ExternalInput")
    b_d = nc.dram_tensor("bmat", [C + 1, GCOLS], f32, kind="ExternalInput")
    v_d = nc.dram_tensor("vmat", [M, C], f32, kind="ExternalInput")
    i_d = nc.dram_tensor("ident", [128, 128], f32, kind="ExternalInput")
    out_d = nc.dram_tensor("out", [N, C], f32, kind="ExternalOutput")
    attn_d = nc.dram_tensor("attn", [N, M], f32, kind="ExternalOutput")

    xr = x_d[:].rearrange("(i j p) c -> i p j c", j=NJ, p=128)
    outr = out_d[:].rearrange("(i j p) c -> i p j c", j=NJ, p=128)
    attnr = attn_d[:].rearrange("(i j p) m -> i p j m", j=NJ, p=128)

    with tile.TileContext(nc) as tc:
        with (
            tc.tile_pool(name="const", bufs=1) as pc,
            tc.tile_pool(name="xin", bufs=3) as px,
            tc.tile_pool(name="xt0", bufs=3) as pxt0,
            tc.tile_pool(name="xt1", bufs=3) as pxt1,
            tc.tile_pool(name="esb", bufs=3) as pe,
            tc.tile_pool(name="atn", bufs=3) as pa,
            tc.tile_pool(name="ats", bufs=3) as pat,
            tc.tile_pool(name="osb", bufs=3) as po,
            tc.tile_pool(name="stat", bufs=2) as pst,
            tc.tile_pool(name="junk", bufs=2) as pjk,
            tc.tile_pool(name="ppt0", bufs=1, space="PSUM") as ppt0,
            tc.tile_pool(name="ppt1", bufs=1, space="PSUM") as ppt1,
            tc.tile_pool(name="ppg", bufs=3, space="PSUM") as ppg,
            tc.tile_pool(name="pat2", bufs=1, space="PSUM") as ppat,
            tc.tile_pool(name="ppo", bufs=2, space="PSUM") as ppo,
        ):
            ident = pc.tile([128, 128], f32)
            nc.sync.dma_start(ident[:], i_d[:])
            b0 = pc.tile([128, GCOLS], f32)
            nc.sync.dma_start(b0[:], b_d[0:128, :])
            b1 = pc.tile([65, GCOLS], f32)
            nc.sync.dma_start(b1[:], b_d[128 : C + 1, :])
            vsb = pc.tile([M, C], f32)
            nc.sync.dma_start(vsb[:], v_d[:])

            for i in range(NBLK):
                # ---- load x block [512, 192] as [128, 4, 192]
                x_sb = px.tile([128, NJ, C], f32)
                nc.sync.dma_start(x_sb[:], xr[i])

                # ---- PE transpose -> xT  (c on partitions)
                pt0 = ppt0.tile([128, NB], f32)
                pt1 = ppt1.tile([64, NB], f32)
                for j in range(NJ):
                    nc.tensor.transpose(pt0[:, ts(j, 128)], x_sb[:, j, 0:128], ident[:])
                    nc.tensor.transpose(pt1[:, ts(j, 128)], x_sb[:, j, 128:C], ident[:])
                xt0 = pxt0.tile([128, NB], f32)
                nc.scalar.copy(xt0[:], pt0[:])
                xt1 = pxt1.tile([65, NB], f32)
                nc.vector.tensor_copy(xt1[0:64, :], pt1[:])
                nc.vector.memset(xt1[64:65, :], 1.0)

                # ---- G = x @ B + bias (one PSUM bank, 4 row-groups)
                pg = ppg.tile([128, NJ, GCOLS], f32)
                for j in range(NJ):
                    nc.tensor.matmul(
                        pg[:, j, :], xt0[:, ts(j, 128)], b0[:], start=True, stop=False
                    )
                    nc.tensor.matmul(
                        pg[:, j, :], xt1[:, ts(j, 128)], b1[:], start=False, stop=True
                    )

                # ---- sum(q^2) per row -> ss[:, j]
                qsq = pjk.tile([128, NJ, DR], f32)
                nc.scalar.activation(qsq[:], pg[:, :, M:GCOLS], AF.Square)
                ss = pst.tile([128, NJ], f32)
                nc.vector.tensor_reduce(
                    ss[:], qsq[:], axis=mybir.AxisListType.X, op=ALU.add
                )
                # rowscale = 1/sqrt(max(ss, eps^2)): Newton-Raphson on DVE
                ssm = pst.tile([128, NJ], f32)
                hn = pst.tile([128, NJ], f32)
                nc.vector.tensor_scalar_max(ssm[:], ss[:], EPS * EPS)
                nc.vector.tensor_scalar(hn[:], ssm[:], -0.5, None, ALU.mult)
                u = pst.tile([128, NJ], i32)
                nc.vector.tensor_scalar(
                    u[:], ssm[:].bitcast(i32), 1, None, ALU.logical_shift_right
                )
                y = pst.tile([128, NJ], f32)
                nc.vector.tensor_scalar(
                    y[:].bitcast(i32), u[:], -1, 0x5F3759DF, ALU.mult, ALU.add
                )
                for _ in range(2):
                    a = pst.tile([128, NJ], f32)
                    nc.vector.tensor_tensor(a[:], y[:], y[:], ALU.mult)
                    b = pst.tile([128, NJ], f32)
                    nc.vector.tensor_tensor(b[:], a[:], hn[:], ALU.mult)
                    y2 = pst.tile([128, NJ], f32)
                    nc.vector.scalar_tensor_tensor(
                        y2[:], b[:], 1.5, y[:], ALU.add, ALU.mult
                    )
                    y = y2
                rsc = y

                # ---- exp(logits * rowscale), fused row-sum -> den
                E = pe.tile([128, NJ, M], f32)
                den = pst.tile([128, NJ], f32)
                for j in range(NJ):
                    nc.scalar.activation(
                        E[:, j, :],
                        pg[:, j, 0:M],
                        AF.Exp,
                        scale=rsc[:, j : j + 1],
                        accum_out=den[:, j : j + 1],
                    )
                rden = pst.tile([128, NJ], f32)
                nc.vector.reciprocal(rden[:], den[:])

                # ---- attn output (normalized)
                attn_sb = pa.tile([128, NJ, M], f32)
                for j in range(NJ):
                    nc.vector.tensor_scalar_mul(
                        attn_sb[:, j, :], E[:, j, :], rden[:, j : j + 1]
                    )
                nc.sync.dma_start(attnr[i], attn_sb[:])

                # ---- attn^T via PE, then out = attn @ v
                pat_ps = ppat.tile([64, NB], f32)
                for j in range(NJ):
                    nc.tensor.transpose(pat_ps[:, ts(j, 128)], E[:, j, :], ident[:])
                at_sb = pat.tile([64, NB], f32)
                nc.scalar.copy(at_sb[:], pat_ps[:])

                o_sb = po.tile([128, NJ, C], f32)
                for j in range(NJ):
                    po_ps = ppo.tile([128, C], f32)
                    nc.tensor.matmul(
                        po_ps[:], at_sb[:, ts(j, 128)], vsb[:], start=True, stop=True
                    )
                    nc.vector.tensor_scalar_mul(
                        o_sb[:, j, :], po_ps[:], rden[:, j : j + 1]
                    )
                nc.sync.dma_start(outr[i], o_sb[:])
    nc.finalize()
    return nc


def _host_prep(td, wq_w, wq_b, wk_w, wk_b, wv_w, wv_b, scale):
    """Per-batch B matrix [193, 74] and v [64, 192], all float32."""
    s = 1.0 + np.clip(scale.astype(np.float32), 0.0, 3.0) * np.float32(
        np.log(NUM_TOKENS)
    )
    s = np.float32(s.reshape(-1)[0])
    bmats, vmats = [], []
    for b in range(td.shape[0]):
        k = td[b] @ wk_w.T + wk_b  # [64, 10]
        knrm = np.sqrt((k * k).sum(-1, keepdims=True))
        kn = k / np.maximum(knrm, EPS)
        skn = s * kn  # fold temperature into keys
        A = wq_w.T @ skn.T  # [192, 64]
        d = skn @ wq_b  # [64]
        Bm = np.zeros((C + 1, GCOLS), dtype=np.float32)
        Bm[0:C, 0:M] = A
        Bm[0:C, M:GCOLS] = wq_w.T
        Bm[C, 0:M] = d
        Bm[C, M:GCOLS] = wq_b
        bmats.append(Bm)
        vmats.append((td[b] @ wv_w.T + wv_b).astype(np.float32))
    return np.stack(bmats), np.stack(vmats)


def kernel(x, td, wq_w, wq_b, wk_w, wk_b, wv_w, wv_b, scale, h, w):
    from concourse.bass_utils import run_bass_kernel_spmd

    x = np.ascontiguousarray(np.asarray(x, dtype=np.float32))
    td = np.asarray(td, dtype=np.float32)
    wq_w = np.asarray(wq_w, dtype=np.float32)
    wq_b = np.asarray(wq_b, dtype=np.float32)
    wk_w = np.asarray(wk_w, dtype=np.float32)
    wk_b = np.asarray(wk_b, dtype=np.float32)
    wv_w = np.asarray(wv_w, dtype=np.float32)
    wv_b = np.asarray(wv_b, dtype=np.float32)
    scale = np.asarray(scale, dtype=np.float32)

    bmats, vmats = _host_prep(td, wq_w, wq_b, wk_w, wk_b, wv_w, wv_b, scale)
    ident = np.eye(128, dtype=np.float32)

    if "nc" not in _cache:
        _cache["nc"] = _build_bass()
    nc = _cache["nc"]

    in_maps = [
        {
            "x": np.ascontiguousarray(x[b]),
            "bmat": np.ascontiguousarray(bmats[b]),
            "vmat": np.ascontiguousarray(vmats[b]),
            "ident": ident,
        }
        for b in range(B_FULL)
    ]
    trace = os.environ.get("KERNEL_TRACE", "0") == "1"
    res = run_bass_kernel_spmd(nc, in_maps, core_ids=list(range(B_FULL)), trace=trace)
    kernel._last_exec_ns = res.exec_time_ns
    kernel._last_results = res
    out = np.stack([r["out"] for r in res.results]).reshape(B_FULL, N, C)
    attn = np.stack([r["attn"] for r in res.results]).reshape(B_FULL, N, M)
    return out, attn


kernel._last_exec_ns = None
kernel._last_results = None
